# revision 1
# baseline (speedup 1.0000x reference)
"""Distributed attention kernel for 8 TRN2 NeuronCores.

Problem: B=2, S=2048, D=1024, H=16 heads (hd=64), no causal mask, no
scaling.  out = softmax((x@Wq) (x@Wk)^T) (x@Wv) @ Wp + biases.

Sharding: DP=2 over batch x TP=4 over heads.  Core c handles batch c//4
and heads 4*(c%4) .. 4*(c%4)+3.  Each core computes its 4 heads'
attention plus the partial c_proj (rows of w_proj for its heads), then a
chunked bf16 ReduceScatter(add) over its 4-core group yields each core's
512-row slice of the final output.  The host reassembles [2,2048,1024].

All matmuls run in bf16 (inputs converted host-side); accumulation f32.
Softmax skips the max-subtraction (scores are O(+-20), exp is safe in
f32): probs = exp(s) / sum exp(s); the denominator comes free as the
65th row of the PV matmul via an appended ones-column on V.
"""

import sys

if "/opt/trn_rl_repo" not in sys.path:
    sys.path.insert(0, "/opt/trn_rl_repo")

import numpy as np
import ml_dtypes

import concourse.bass as bass
import concourse.mybir as mybir
from concourse import bacc
from concourse.tile import TileContext
from concourse.bass_utils import run_bass_kernel_spmd

BF16 = mybir.dt.bfloat16
F32 = mybir.dt.float32

B, S, D = 2, 2048, 1024
H = 16
HD = 64
TP = 4  # tensor-parallel group size (cores per batch)
HPC = H // TP  # heads per core = 4
QC = HPC * HD  # q (or k or v) columns per core = 256
SQB = 512  # sq chunk (free dim of scores/pv matmuls)
NJ = S // SQB  # 4 chunks
NT = S // 128  # 16 sk tiles
NK = D // 128  # 8 contraction tiles for the projections
SO = S // TP  # 512 output rows per core

_CACHE = {}


def build():
    nc = bacc.Bacc(num_devices=8)

    xT_ext = nc.declare_dram_parameter("xT", [D, S], BF16, isOutput=False)
    wqkv_ext = nc.declare_dram_parameter("wqkv", [D, 3 * QC], BF16, isOutput=False)
    bqk_ext = nc.declare_dram_parameter("bqk", [2 * QC, 1], F32, isOutput=False)
    bv_ext = nc.declare_dram_parameter("bv", [1, QC], BF16, isOutput=False)
    wpa_ext = nc.declare_dram_parameter("wpa", [QC + 1, D], BF16, isOutput=False)
    out_ext = nc.declare_dram_parameter("out", [SO, D], BF16, isOutput=True)

    partial = nc.dram_tensor("partial", [S, D], BF16)
    # 8 reduce-scatter pieces of 256 rows each -> [64, D] per core
    NR = 2 * NJ
    RROWS = S // NR  # 256
    rs_out = [nc.dram_tensor(f"rs_out{r}", [RROWS // TP, D], BF16) for r in range(NR)]

    with TileContext(nc) as tc:
        with (
            tc.tile_pool(name="persist", bufs=1) as persist,
            tc.tile_pool(name="expt_pool", bufs=3) as expt_pool,
            tc.tile_pool(name="mm", bufs=3, space="PSUM") as mm_pool,
            tc.tile_pool(name="pv", bufs=2, space="PSUM") as pv_pool,
            tc.tile_pool(name="small", bufs=4) as small_pool,
            tc.tile_pool(name="ot", bufs=4) as ot_pool,
            tc.tile_pool(name="osb", bufs=6) as osb_pool,
        ):
            # ---- load persistent tiles ----
            # Interleave w/x loads in k order so the first qkv matmul
            # (k=0) can start as soon as the first pair lands.
            xt = []
            wt = []
            for k in range(NK):
                tw = persist.tile([128, 3 * QC], BF16, tag=f"wt{k}", name=f"wt{k}")
                nc.sync.dma_start(out=tw, in_=wqkv_ext[k * 128 : (k + 1) * 128, :])
                wt.append(tw)
                tx = persist.tile([128, S], BF16, tag=f"xt{k}", name=f"xt{k}")
                nc.sync.dma_start(out=tx, in_=xT_ext[k * 128 : (k + 1) * 128, :])
                xt.append(tx)
            wp = []
            for p in range(2):
                t = persist.tile([128, D], BF16, tag=f"wp{p}", name=f"wp{p}")
                nc.sync.dma_start(out=t, in_=wpa_ext[p * 128 : (p + 1) * 128, :])
                wp.append(t)
            wp_bias = persist.tile([1, D], BF16, tag="wpb", name="wpb")
            nc.sync.dma_start(out=wp_bias, in_=wpa_ext[2 * 128 : 2 * 128 + 1, :])
            bqk = []
            for k in range(4):
                t = persist.tile([128, 1], F32, tag=f"bqk{k}", name=f"bqk{k}")
                nc.sync.dma_start(out=t, in_=bqk_ext[k * 128 : (k + 1) * 128, :])
                bqk.append(t)
            bv = persist.tile([1, QC], BF16, tag="bv", name="bv")
            nc.sync.dma_start(out=bv, in_=bv_ext[:, :])
            ones_row = persist.tile([1, 128], BF16, tag="ones", name="ones")
            nc.vector.memset(ones_row, 1.0)

            # ---- QKV projection ----
            # q/k transposed layout: qk_sb[ct] [128, S], ct 0-1 = q cols,
            # ct 2-3 = k cols; head h lives on partitions (h%2)*64 of
            # tile h//2 (+2 for k).  Emit k first so attention can start
            # before the q tiles for later chunks are done.
            qk_sb = [
                persist.tile([128, S], BF16, tag=f"qk{ct}", name=f"qk{ct}")
                for ct in range(4)
            ]
            def qkv_col_tile(ct, ns):
                ps = mm_pool.tile([128, 2, SQB], F32, tag="mm", name="ps_qkv")
                for k in range(NK):
                    nc.tensor.matmul(
                        ps[:, 0, :],
                        wt[k][:, ct * 128 : (ct + 1) * 128],
                        xt[k][:, ns * SQB : (ns + 1) * SQB],
                        start=(k == 0),
                        stop=(k == NK - 1),
                    )
                nc.vector.tensor_scalar_add(
                    qk_sb[ct][:, ns * SQB : (ns + 1) * SQB], ps[:, 0, :], bqk[ct]
                )

            # k tiles first (attention needs the full kT), v next, then q
            # chunk-major so chunk 0's attention can start early.
            for ct in (2, 3):
                for ns in range(NJ):
                    qkv_col_tile(ct, ns)

            # v natural layout + ones column: v_sb[t] [128, HPC, 65];
            # [:, h, :64] = v for head h, [:, h, 64] = 1.0
            v_sb = []
            for t_i in range(NT):
                t = persist.tile(
                    [128, HPC, HD + 1], BF16, tag=f"v{t_i}", name=f"v{t_i}"
                )
                v_sb.append(t)
            for t_i in range(NT):
                psv = mm_pool.tile([128, 2, SQB], F32, tag="mm", name="ps_v")
                for k in range(NK):
                    nc.tensor.matmul(
                        psv[:, 0, 0:QC],
                        xt[k][:, t_i * 128 : (t_i + 1) * 128],
                        wt[k][:, 2 * QC : 3 * QC],
                        start=(k == 0),
                        stop=False,
                    )
                nc.tensor.matmul(psv[:, 0, 0:QC], ones_row, bv, start=False, stop=True)
                nc.vector.memset(v_sb[t_i][:, :, HD : HD + 1], 1.0)
                for h in range(HPC):
                    nc.vector.tensor_copy(
                        v_sb[t_i][:, h, 0:HD], psv[:, 0, h * HD : (h + 1) * HD]
                    )
            # q tiles for chunk 0 only; later chunks' q tiles are emitted
            # as PE filler inside the attention pipeline.
            for ct in (0, 1):
                qkv_col_tile(ct, 0)

            # ---- attention + c_proj + reduce-scatter, head-pipelined ----
            # Stage A(j,h): scores + exp.  Stage B(j,h): pv + normalize.
            # The attention inner loop is ACT(exp)-bound, so the in-order
            # PE queue must always hold ready work or the HAM clock gate
            # halves the PE clock.  After every stage we pop a few
            # "filler" groups (q projection for later chunks, c_proj
            # pieces for the previous chunk) that are ready to run.
            fillers = []

            def emit_fillers(n):
                for _ in range(n):
                    if fillers:
                        fillers.pop(0)()

            def normalize(h, pv, om):
                rz = small_pool.tile([1, SQB], F32, tag="rz", name="rz")
                nc.vector.reciprocal(rz, pv[HD : HD + 1, :])
                bc = small_pool.tile([HD, SQB], F32, tag="bc", name="bc")
                nc.gpsimd.partition_broadcast(bc, rz)
                if h % 2 == 0:
                    nc.vector.tensor_mul(om[h // 2][0:HD, :], pv[0:HD, :], bc)
                else:
                    o = ot_pool.tile([HD, SQB], BF16, tag="ot", name="ot")
                    nc.vector.tensor_mul(o, pv[0:HD, :], bc)
                    nc.sync.dma_start(out=om[h // 2][HD:128, :], in_=o)

            def stage_ab(j, h, prev):
                """Scores+exp for (j,h) interleaved with the pv matmuls of
                the previous head `prev` = (pj, ph, expt, om) or None.
                The pv matmuls need no scores-psum slot, so the PE always
                has ready work while ACT drains the scores banks."""
                expt = None
                if h is not None:
                    qslice = qk_sb[h // 2][
                        (h % 2) * HD : (h % 2) * HD + HD, j * SQB : (j + 1) * SQB
                    ]
                    krow = qk_sb[2 + h // 2][(h % 2) * HD : (h % 2) * HD + HD, :]
                    expt = expt_pool.tile(
                        [128, NT, SQB], BF16, tag="expt", name="expt"
                    )
                pvp = None
                if prev is not None:
                    pj, ph, pexpt, pom = prev
                    pvp = pv_pool.tile([HD + 1, SQB], F32, tag="pv", name="pv")
                for t2 in range(NT // 2):
                    if h is not None:
                        ps_s = mm_pool.tile(
                            [128, 2, SQB], F32, tag="mm", name="ps_s"
                        )
                        for u in range(2):
                            t_i = 2 * t2 + u
                            nc.tensor.matmul(
                                ps_s[:, u, :],
                                krow[:, t_i * 128 : (t_i + 1) * 128],
                                qslice,
                                start=True,
                                stop=True,
                            )
                        nc.scalar.activation(
                            expt[:, 2 * t2 : 2 * t2 + 2, :],
                            ps_s,
                            mybir.ActivationFunctionType.Exp,
                        )
                    if prev is not None:
                        for u in range(2):
                            t_i = 2 * t2 + u
                            nc.tensor.matmul(
                                pvp,
                                v_sb[t_i][:, ph, :],
                                pexpt[:, t_i, :],
                                start=(t_i == 0),
                                stop=(t_i == NT - 1),
                            )
                if prev is not None:
                    normalize(ph, pvp, pom)
                return expt

            def cproj_piece(j, om, m, nch):
                pc = mm_pool.tile([128, 2, SQB], F32, tag="mm", name="pc")
                for p in range(2):
                    nc.tensor.matmul(
                        pc[:, 0, :],
                        om[p][:, m * 128 : (m + 1) * 128],
                        wp[p][:, nch * 512 : (nch + 1) * 512],
                        start=(p == 0),
                        stop=False,
                    )
                nc.tensor.matmul(
                    pc[:, 0, :],
                    ones_row,
                    wp_bias[:, nch * 512 : (nch + 1) * 512],
                    start=False,
                    stop=True,
                )
                osb = osb_pool.tile([128, 512], BF16, tag="osb", name="osb")
                nc.vector.tensor_copy(osb, pc[:, 0, :])
                nc.sync.dma_start(
                    out=partial[
                        j * SQB + m * 128 : j * SQB + (m + 1) * 128,
                        nch * 512 : (nch + 1) * 512,
                    ],
                    in_=osb,
                )

            def rs_piece(r):
                nc.gpsimd.collective_compute(
                    "ReduceScatter",
                    mybir.AluOpType.add,
                    replica_groups=[[0, 1, 2, 3], [4, 5, 6, 7]],
                    ins=[partial[r * RROWS : (r + 1) * RROWS, :]],
                    outs=[rs_out[r].ap()],
                )
                nc.sync.dma_start(
                    out=out_ext[r * (RROWS // TP) : (r + 1) * (RROWS // TP), :],
                    in_=rs_out[r][:, :],
                )

            def enqueue_cproj(j, om):
                for m in range(SQB // 128):
                    for nch in range(2):
                        fillers.append(
                            lambda j=j, om=om, m=m, nch=nch: cproj_piece(j, om, m, nch)
                        )
                    if m == 1:
                        fillers.append(lambda r=2 * j: rs_piece(r))
                fillers.append(lambda r=2 * j + 1: rs_piece(r))

            om_of = {}
            prev = None  # (pj, ph, expt, om) pending pv/normalize
            for j in range(NJ):
                om_of[j] = [
                    ot_pool.tile([128, SQB], BF16, tag="om", name=f"om{p}")
                    for p in range(2)
                ]
                for h in range(HPC):
                    expt = stage_ab(j, h, prev)
                    prev = (j, h, expt, om_of[j])
                    # q tiles for the NEXT chunk at fixed positions so
                    # they always precede that chunk's scores in program
                    # order; c_proj/RS pieces fill the remaining slots.
                    if h < 2 and j + 1 < NJ:
                        qkv_col_tile(h, j + 1)
                        emit_fillers(1)
                    else:
                        # drain fillers faster late in the chunk so the
                        # RS pieces reach the CC queue before the tail
                        emit_fillers(3)
                    if j > 0 and h == 0:
                        enqueue_cproj(j - 1, om_of[j - 1])
            # drain: pv/normalize for the last head, last cproj/RS
            stage_ab(None, None, prev)
            enqueue_cproj(NJ - 1, om_of[NJ - 1])
            emit_fillers(len(fillers))

    nc.compile()
    return nc


def make_in_maps(x, w_attn, b_attn, w_proj, b_proj):
    bf = ml_dtypes.bfloat16
    in_maps = []
    for c in range(8):
        b = c // TP
        g = c % TP
        cs = slice(g * QC, (g + 1) * QC)
        xT = np.ascontiguousarray(x[b].T).astype(bf)
        wqkv = np.concatenate(
            [w_attn[:, cs], w_attn[:, D:][:, cs], w_attn[:, 2 * D :][:, cs]], axis=1
        ).astype(bf)
        bqk = np.concatenate([b_attn[cs], b_attn[D:][cs]]).reshape(2 * QC, 1)
        bqk = np.ascontiguousarray(bqk, dtype=np.float32)
        bv = np.ascontiguousarray(b_attn[2 * D :][cs].reshape(1, QC).astype(bf))
        wpa = np.concatenate(
            [w_proj[cs, :], (b_proj / TP).reshape(1, D)], axis=0
        ).astype(bf)
        in_maps.append({"xT": xT, "wqkv": wqkv, "bqk": bqk, "bv": bv, "wpa": wpa})
    return in_maps


def assemble(results):
    # Chunk j's reduce-scatter gives core (group rank g) rows
    # j*SQB + g*128 .. +128; the kernel writes them to out rows j*128..,
    # so core c's "out" holds rows {j*SQB + g*128 + r} for j in 0..3.
    out = np.empty((B, S, D), np.float32)
    NR = 2 * NJ
    RROWS = S // NR  # 256 partial rows per RS piece
    W = RROWS // TP  # 64 output rows per core per piece
    for c in range(8):
        b = c // TP
        g = c % TP
        o = np.asarray(results[c]["out"]).astype(np.float32)
        for r in range(NR):
            out[b, r * RROWS + g * W : r * RROWS + (g + 1) * W, :] = o[
                r * W : (r + 1) * W
            ]
    return out


def kernel(x, w_attn, b_attn, w_proj, b_proj):
    x = np.asarray(x, dtype=np.float32)
    w_attn = np.asarray(w_attn, dtype=np.float32)
    b_attn = np.asarray(b_attn, dtype=np.float32)
    w_proj = np.asarray(w_proj, dtype=np.float32)
    b_proj = np.asarray(b_proj, dtype=np.float32)
    if "nc" not in _CACHE:
        _CACHE["nc"] = build()
    nc = _CACHE["nc"]
    in_maps = make_in_maps(x, w_attn, b_attn, w_proj, b_proj)
    res = run_bass_kernel_spmd(nc, in_maps, core_ids=list(range(8)))
    return assemble(res.results)



# revision 7
# speedup vs baseline: 1.2281x; 1.2281x over previous
"""Distributed attention kernel for 8 TRN2 NeuronCores (v3).

Problem: B=2, S=2048, D=1024, H=16 heads (hd=64), no causal mask, no
scaling.  out = softmax((x@Wq) (x@Wk)^T) (x@Wv) @ Wp + biases.

Sharding: DP=2 over batch x TP=4 over heads.  Core c handles batch c//4
and heads 4*(c%4) .. 4*(c%4)+3.  Each core computes its 4 heads'
attention in 512-q-row chunks; per chunk, two small AllGathers (one per
2-head om tile, 128KB each) give every core in the group the full
[1024 hd, 512 q] normalized attention block, and each core runs c_proj
for its OWN 256-column slice of w_proj (column-parallel, no reduction
needed).  vs a trailing ReduceScatter of f32 partials this cuts
collective bytes ~3x, removes the partial adds, and the half-chunk
granularity lets the final c_proj overlap the last gather.

All matmuls bf16 (host-side cast), f32 accumulation.  Softmax skips the
max-subtraction (scores are O(+-25), exp is safe in f32); the
denominator comes free as the 65th row of the PV matmul via an appended
ones-column on V.

Scheduling notes:
- QKV runs k-major over 8 (then 16) concurrent PSUM accumulators so the
  PE keeps pace with the initial HBM load of x.
- Scores PSUM tiles hold 3 k-tiles (3 banks); each exp instruction
  covers [128, 1536] -- fewer ACT instructions, less fixed overhead
  (the ACT engine is the pacing engine in steady state).
- Biases fold into DVE copies, not PE matmuls.
- softmax denominators use reciprocal_approx_fast (~5x faster than
  InstReciprocal; 18 good bits is plenty).
"""

import sys

if "/opt/trn_rl_repo" not in sys.path:
    sys.path.insert(0, "/opt/trn_rl_repo")

import numpy as np
import ml_dtypes

import concourse.bass as bass
import concourse.mybir as mybir
from concourse import bacc
from concourse.tile import TileContext
from concourse.bass_utils import run_bass_kernel_spmd

BF16 = mybir.dt.bfloat16
F32 = mybir.dt.float32

B, S, D = 2, 2048, 1024
H = 16
HD = 64
TP = 4  # tensor-parallel group size (cores per batch)
HPC = H // TP  # heads per core = 4
QC = HPC * HD  # q (or k or v) columns per core = 256
OCW = D // TP  # c_proj output columns per core = 256
SQB = 512  # q chunk (free dim of scores/pv matmuls)
NJ = S // SQB  # 4 chunks
NT = S // 128  # 16 sk tiles
NK = D // 128  # 8 contraction tiles for the projections

_CACHE = {}


def build():
    nc = bacc.Bacc(num_devices=8)

    xT_ext = nc.declare_dram_parameter("xT", [D, S], BF16, isOutput=False)
    wqkv_ext = nc.declare_dram_parameter("wqkv", [D, 3 * QC], BF16, isOutput=False)
    bqk_ext = nc.declare_dram_parameter("bqk", [2 * QC, 1], F32, isOutput=False)
    bv_ext = nc.declare_dram_parameter("bv", [1, QC], F32, isOutput=False)
    wpc_ext = nc.declare_dram_parameter("wpc", [D, OCW], BF16, isOutput=False)
    bpc_ext = nc.declare_dram_parameter("bpc", [1, OCW], F32, isOutput=False)
    out_ext = nc.declare_dram_parameter("out", [S, OCW], BF16, isOutput=True)

    groups = [[0, 1, 2, 3], [4, 5, 6, 7]]
    ag_in = [
        [nc.dram_tensor(f"ag_in{j}_{p}", [128, SQB], BF16) for p in range(2)]
        for j in range(NJ)
    ]
    ag_out = [
        [nc.dram_tensor(f"ag_out{j}_{p}", [TP, 128, SQB], BF16) for p in range(2)]
        for j in range(NJ)
    ]

    with TileContext(nc) as tc:
        with (
            tc.tile_pool(name="persist", bufs=1) as persist,
            tc.tile_pool(name="expt_pool", bufs=3) as expt_pool,
            tc.tile_pool(name="ps", bufs=2, space="PSUM") as ps,
            tc.tile_pool(name="small", bufs=4) as small_pool,
            tc.tile_pool(name="ot", bufs=4) as ot_pool,
            tc.tile_pool(name="osb", bufs=6) as osb_pool,
            tc.tile_pool(name="attp", bufs=16) as att_pool,
        ):
            # ---- persistent loads (interleaved so QKV can start early) ----
            xt = []
            wt = []
            for k in range(NK):
                tw = persist.tile([128, 3 * QC], BF16, tag=f"wt{k}", name=f"wt{k}")
                nc.sync.dma_start(out=tw, in_=wqkv_ext[k * 128 : (k + 1) * 128, :])
                wt.append(tw)
                tx = persist.tile([128, S], BF16, tag=f"xt{k}", name=f"xt{k}")
                nc.sync.dma_start(out=tx, in_=xT_ext[k * 128 : (k + 1) * 128, :])
                xt.append(tx)
            bqk = []
            for k in range(4):
                t = persist.tile([128, 1], F32, tag=f"bqk{k}", name=f"bqk{k}")
                nc.sync.dma_start(out=t, in_=bqk_ext[k * 128 : (k + 1) * 128, :])
                bqk.append(t)
            bv = persist.tile([1, QC], F32, tag="bv", name="bv")
            nc.sync.dma_start(out=bv, in_=bv_ext[:, :])
            bpc = persist.tile([1, OCW], F32, tag="bpc", name="bpc")
            nc.sync.dma_start(out=bpc, in_=bpc_ext[:, :])
            wp = []
            for r in range(NK):
                t = persist.tile([128, OCW], BF16, tag=f"wp{r}", name=f"wp{r}")
                nc.sync.dma_start(out=t, in_=wpc_ext[r * 128 : (r + 1) * 128, :])
                wp.append(t)
            # broadcast bias rows across partitions for the DVE folds
            vb_b = persist.tile([128, QC], F32, tag="vb_b", name="vb_b")
            nc.gpsimd.partition_broadcast(vb_b, bv)
            pb_b = persist.tile([128, OCW], F32, tag="pb_b", name="pb_b")
            nc.gpsimd.partition_broadcast(pb_b, bpc)

            # v natural layout + ones column: v_sb[t] [128, HPC, 65]
            v_sb = []
            for t_i in range(NT):
                t = persist.tile(
                    [128, HPC, HD + 1], BF16, tag=f"v{t_i}", name=f"v{t_i}"
                )
                v_sb.append(t)
                nc.vector.memset(t[:, :, HD : HD + 1], 1.0)

            # q/k transposed layout: qk_sb[ct] [128, S]; ct 0-1 = q cols,
            # ct 2-3 = k cols; head h on partitions (h%2)*64 of tile h//2.
            qk_sb = [
                persist.tile([128, S], BF16, tag=f"qk{ct}", name=f"qk{ct}")
                for ct in range(4)
            ]

            # ---- wave A: k columns, k-major over 8 concurrent psums ----
            scA = [
                ps.tile([128, 3, SQB], F32, tag="sc", name=f"scA{i}") for i in range(2)
            ]
            pvA = [
                ps.tile([128, SQB], F32, tag="pv", name=f"pvA{i}") for i in range(2)
            ]
            wa = [(2, 0), (2, 1), (2, 2), (2, 3), (3, 0), (3, 1), (3, 2), (3, 3)]
            wa_aps = [
                scA[0][:, 0, :], scA[0][:, 1, :], scA[0][:, 2, :],
                scA[1][:, 0, :], scA[1][:, 1, :], scA[1][:, 2, :],
                pvA[0], pvA[1],
            ]
            for k in range(NK):
                for (ct, ns), ap in zip(wa, wa_aps):
                    nc.tensor.matmul(
                        ap,
                        wt[k][:, ct * 128 : (ct + 1) * 128],
                        xt[k][:, ns * SQB : (ns + 1) * SQB],
                        start=(k == 0),
                        stop=(k == NK - 1),
                    )
            for (ct, ns), ap in zip(wa, wa_aps):
                nc.vector.tensor_scalar_add(
                    qk_sb[ct][:, ns * SQB : (ns + 1) * SQB], ap, bqk[ct]
                )

            # ---- q columns for one chunk (chunk 0 now, others as fillers) --
            def qcols(ns):
                t = ps.tile([128, 3, SQB], F32, tag="sc", name="qcols")
                for k in range(NK):
                    for ct in range(2):
                        nc.tensor.matmul(
                            t[:, ct, :],
                            wt[k][:, ct * 128 : (ct + 1) * 128],
                            xt[k][:, ns * SQB : (ns + 1) * SQB],
                            start=(k == 0),
                            stop=(k == NK - 1),
                        )
                for ct in range(2):
                    nc.vector.tensor_scalar_add(
                        qk_sb[ct][:, ns * SQB : (ns + 1) * SQB], t[:, ct, :], bqk[ct]
                    )

            qcols(0)

            # ---- wave B: v projection, k-major over 8 concurrent psums ----
            # One accumulator per 2KB psum bank: a matmul's start=True
            # zeroes the whole bank ("zero region"), so two 1KB
            # accumulators must not share one.  Two rounds of 8.
            def wave_v_half(r0):
                vt = [
                    ps.tile([128, 3, SQB], F32, tag="sc", name=f"vB{i}")
                    for i in range(2)
                ]
                vp = [
                    ps.tile([128, SQB], F32, tag="pv", name=f"vP{i}")
                    for i in range(2)
                ]
                vaps = [vt[i][:, u, 0:256] for i in range(2) for u in range(3)] + [
                    vp[i][:, 0:256] for i in range(2)
                ]
                for k in range(NK):
                    for u, ap in enumerate(vaps):
                        tt = r0 + u
                        nc.tensor.matmul(
                            ap,
                            xt[k][:, tt * 128 : (tt + 1) * 128],
                            wt[k][:, 2 * QC : 3 * QC],
                            start=(k == 0),
                            stop=(k == NK - 1),
                        )
                for u, ap in enumerate(vaps):
                    tt = r0 + u
                    for hh in range(HPC):
                        nc.vector.tensor_add(
                            v_sb[tt][:, hh, 0:HD],
                            ap[:, hh * HD : (hh + 1) * HD],
                            vb_b[:, hh * HD : (hh + 1) * HD],
                        )

            def wave_v():
                wave_v_half(0)
                wave_v_half(8)

            # ---- attention pipeline ----
            # Stage (j,h): scores+exp for (j,h) interleaved with the pv
            # matmuls of the previous head (keeps the PE queue full while
            # ACT drains the score banks), then normalize(prev).
            def normalize(ph, pvp, pom):
                # copy z to a partition-0 tile first: the custom-DVE
                # reciprocal_approx_fast misreads inputs based at
                # partition 64 (native ops handle the shift fine)
                zrow = small_pool.tile([1, SQB], F32, tag="zrow", name="zrow")
                nc.vector.tensor_copy(zrow, pvp[HD : HD + 1, :])
                rz = small_pool.tile([1, SQB], F32, tag="rz", name="rz")
                nc.vector.reciprocal_approx_fast(rz, zrow)
                bc = small_pool.tile([HD, SQB], F32, tag="bc", name="bc")
                nc.gpsimd.partition_broadcast(bc, rz)
                if ph % 2 == 0:
                    nc.vector.tensor_mul(pom[ph // 2][0:HD, :], pvp[0:HD, :], bc)
                else:
                    o = ot_pool.tile([HD, SQB], BF16, tag="ot", name="ot")
                    nc.vector.tensor_mul(o, pvp[0:HD, :], bc)
                    nc.sync.dma_start(out=pom[ph // 2][HD:128, :], in_=o)

            GRP = [(0, 3), (3, 3), (6, 3), (9, 3), (12, 2), (14, 2)]

            def stage_ab(j, h, prev):
                expt = None
                if h is not None:
                    qslice = qk_sb[h // 2][
                        (h % 2) * HD : (h % 2) * HD + HD, j * SQB : (j + 1) * SQB
                    ]
                    krow = qk_sb[2 + h // 2][(h % 2) * HD : (h % 2) * HD + HD, :]
                    expt = expt_pool.tile(
                        [128, NT, SQB], BF16, tag="expt", name="expt"
                    )
                pvp = None
                if prev is not None:
                    pj, ph, pexpt, pom = prev
                    pvp = ps.tile([HD + 1, SQB], F32, tag="pv", name="pv")
                pv_t = [0]

                def emit_pv(n):
                    if prev is None:
                        return
                    while n > 0 and pv_t[0] < NT:
                        t_i = pv_t[0]
                        nc.tensor.matmul(
                            pvp,
                            v_sb[t_i][:, ph, :],
                            pexpt[:, t_i, :],
                            start=(t_i == 0),
                            stop=(t_i == NT - 1),
                        )
                        pv_t[0] += 1
                        n -= 1

                for t0, glen in GRP:
                    if h is not None:
                        ps_s = ps.tile([128, 3, SQB], F32, tag="sc", name="ps_s")
                        for u in range(glen):
                            nc.tensor.matmul(
                                ps_s[:, u, :],
                                krow[:, (t0 + u) * 128 : (t0 + u + 1) * 128],
                                qslice,
                                start=True,
                                stop=True,
                            )
                        nc.scalar.activation(
                            expt[:, t0 : t0 + glen, :],
                            ps_s[:, 0:glen, :],
                            mybir.ActivationFunctionType.Exp,
                        )
                    emit_pv(3)
                emit_pv(NT)
                if prev is not None:
                    normalize(ph, pvp, pom)
                return expt

            # ---- allgather + column-parallel c_proj per chunk ----
            att_of = {}

            def send_ag(j, p, om):
                nc.sync.dma_start(out=ag_in[j][p][:, :], in_=om[p][:, :])
                nc.gpsimd.collective_compute(
                    "AllGather",
                    mybir.AluOpType.bypass,
                    replica_groups=groups,
                    ins=[ag_in[j][p].ap()],
                    outs=[ag_out[j][p].ap()],
                )

            def recv_att(j, p):
                tiles = []
                for sr in range(TP):
                    t = att_pool.tile(
                        [128, SQB], BF16, tag="attw", name=f"att{sr}_{p}"
                    )
                    nc.sync.dma_start(out=t, in_=ag_out[j][p][sr, :, :])
                    tiles.append(t)
                att_of[(j, p)] = tiles

            def cproj(j):
                # out[q, oc-slice] for the whole 512-q chunk, 4 q-tiles.
                # Within each accumulation group run the p=0 (heads 0-1 of
                # each peer) matmuls first so the group can start before
                # the second gather has landed.
                pcs = []
                t1 = ps.tile([128, 3, SQB], F32, tag="sc", name="pc1")
                t2 = ps.tile([128, 3, SQB], F32, tag="sc", name="pc2")
                for qq in range(4):
                    pcs.append((t1 if qq < 3 else t2)[:, qq % 3, 0:OCW])
                for p in range(2):
                    for qq in range(4):
                        for sr in range(TP):
                            r = sr * 2 + p
                            nc.tensor.matmul(
                                pcs[qq],
                                att_of[(j, p)][sr][:, qq * 128 : (qq + 1) * 128],
                                wp[r],
                                start=(p == 0 and sr == 0),
                                stop=(p == 1 and sr == TP - 1),
                            )
                for qq in range(4):
                    osb = osb_pool.tile([128, OCW], BF16, tag="osb", name="osb")
                    nc.vector.tensor_add(osb, pcs[qq], pb_b)
                    nc.sync.dma_start(
                        out=out_ext[j * SQB + qq * 128 : j * SQB + (qq + 1) * 128, :],
                        in_=osb,
                    )

            om_of = {}
            prev = None
            for j in range(NJ):
                om_of[j] = [
                    ot_pool.tile([128, SQB], BF16, tag="om", name=f"om{p}", bufs=4)
                    for p in range(2)
                ]
                for h in range(HPC):
                    expt = stage_ab(j, h, prev)
                    prev = (j, h, expt, om_of[j])
                    if h == 0:
                        if j == 0:
                            wave_v()
                        else:
                            send_ag(j - 1, 1, om_of[j - 1])
                    elif h == 1:
                        if j + 1 < NJ:
                            qcols(j + 1)
                        if j >= 1:
                            recv_att(j - 1, 0)
                    elif h == 2:
                        send_ag(j, 0, om_of[j])
                        if j >= 1:
                            recv_att(j - 1, 1)
                    elif h == 3:
                        if j >= 1:
                            cproj(j - 1)
            # drain: pv/normalize for the last head, then the last chunk's
            # second gather + c_proj
            stage_ab(None, None, prev)
            send_ag(NJ - 1, 1, om_of[NJ - 1])
            recv_att(NJ - 1, 0)
            recv_att(NJ - 1, 1)
            cproj(NJ - 1)

    nc.compile()
    return nc


def make_in_maps(x, w_attn, b_attn, w_proj, b_proj):
    bf = ml_dtypes.bfloat16
    in_maps = []
    for c in range(8):
        b = c // TP
        g = c % TP
        cs = slice(g * QC, (g + 1) * QC)
        ocs = slice(g * OCW, (g + 1) * OCW)
        xT = np.ascontiguousarray(x[b].T).astype(bf)
        wqkv = np.concatenate(
            [w_attn[:, cs], w_attn[:, D:][:, cs], w_attn[:, 2 * D :][:, cs]], axis=1
        ).astype(bf)
        bqk = np.concatenate([b_attn[cs], b_attn[D:][cs]]).reshape(2 * QC, 1)
        bqk = np.ascontiguousarray(bqk, dtype=np.float32)
        bv = np.ascontiguousarray(
            b_attn[2 * D :][cs].reshape(1, QC), dtype=np.float32
        )
        wpc = np.ascontiguousarray(w_proj[:, ocs]).astype(bf)
        bpc = np.ascontiguousarray(b_proj[ocs].reshape(1, OCW), dtype=np.float32)
        in_maps.append(
            {"xT": xT, "wqkv": wqkv, "bqk": bqk, "bv": bv, "wpc": wpc, "bpc": bpc}
        )
    return in_maps


def assemble(results):
    # Core (b, g) owns output columns g*OCW..(g+1)*OCW for all of batch b.
    out = np.empty((B, S, D), np.float32)
    for c in range(8):
        b = c // TP
        g = c % TP
        o = np.asarray(results[c]["out"]).astype(np.float32)
        out[b, :, g * OCW : (g + 1) * OCW] = o
    return out


def kernel(x, w_attn, b_attn, w_proj, b_proj):
    x = np.asarray(x, dtype=np.float32)
    w_attn = np.asarray(w_attn, dtype=np.float32)
    b_attn = np.asarray(b_attn, dtype=np.float32)
    w_proj = np.asarray(w_proj, dtype=np.float32)
    b_proj = np.asarray(b_proj, dtype=np.float32)
    if "nc" not in _CACHE:
        _CACHE["nc"] = build()
    nc = _CACHE["nc"]
    in_maps = make_in_maps(x, w_attn, b_attn, w_proj, b_proj)
    res = run_bass_kernel_spmd(nc, in_maps, core_ids=list(range(8)))
    return assemble(res.results)
